# revision 35
# baseline (speedup 1.0000x reference)
"""Trainium2 Bass kernel for nn_AttCNN4Weight (sparse_attention).

Data-parallel over batch: each of the 8 NeuronCores handles 8 of the 64
batch elements end-to-end (dynamic per-sample conv kernel -> sliding-window
score -> masked softmax over kv_len -> weighted sum of v). No collectives.

Host-side work is layout only: batch sharding, transposes so every DMA
moves multi-KB contiguous rows, a column reorder of W to (tap, channel)
order, and a bf16 cast of v (the attend reduction tolerates bf16 easily;
halves v HBM traffic).

Performance structure (memory-bound problem, ~51MB/core HBM traffic):
- f32r single-pass matmuls for the score conv (fp32 is 2 half-rate passes).
- KW=3 taps come out of ONE matmul pass (M=3); the +/-1 tap shifts are
  resolved by free-dim offsets after a DMA partition-scatter into a
  per-batch staging row (engine SBUF access must start at partition
  0/32/64/96, so cross-partition landing goes through DMA).
- Three DMA issue rings: sync carries the big k/v input streams in demand
  order, scalar carries params + compute-dependent stores (so the input
  stream never head-of-line blocks on compute), gpsimd is free.
- The batch is processed in 2 groups of 4: group 0's softmax+attend
  overlap group 1's k streaming, leaving only the last group's short
  serial chain exposed at the end.
"""

import sys

if "/opt/trn_rl_repo" not in sys.path:
    sys.path.insert(0, "/opt/trn_rl_repo")

import numpy as np
from contextlib import ExitStack

L, B, C, Q, V, KW = 2048, 64, 512, 512, 512, 3
NCORES = 8
BC = B // NCORES          # 8 batch elements per core
G = 2                     # batch groups per core
BG = BC // G              # 4 batch elements per group
M12 = KW * (C // 128)     # 12 contraction chunks of (tap, channel)
NEGBIG = 3.0e38           # additive mask constant (finite, exp() underflows to 0)

_NC = None


def _build():
    import concourse.bacc as bacc
    import concourse.tile as tile
    from concourse import mybir
    from concourse.masks import make_identity

    f32 = mybir.dt.float32
    f32r = mybir.dt.float32r
    bf16 = mybir.dt.bfloat16
    i32 = mybir.dt.int32

    nc = bacc.Bacc(None)

    kT = nc.declare_dram_parameter("kT", [BC, C, L], f32, isOutput=False)
    vT = nc.declare_dram_parameter("vT", [BC, L, V], bf16, isOutput=False)
    mT = nc.declare_dram_parameter("mT", [BC, L], i32, isOutput=False)
    qT = nc.declare_dram_parameter("qT", [Q, BC], f32, isOutput=False)
    Wr = nc.declare_dram_parameter("Wr", [Q, KW * C], f32, isOutput=False)
    Br = nc.declare_dram_parameter("Br", [128, M12], f32, isOutput=False)
    a_out = nc.declare_dram_parameter("a_out", [BC, L], f32, isOutput=True)
    e_out = nc.declare_dram_parameter("e_out", [BC, L], f32, isOutput=True)
    t_out = nc.declare_dram_parameter("t_out", [BC, V], f32, isOutput=True)

    with ExitStack() as ctx:
        tc = ctx.enter_context(tile.TileContext(nc))
        singles = ctx.enter_context(tc.tile_pool(name="singles", bufs=1))
        sa = ctx.enter_context(tc.tile_pool(name="sa", bufs=2))
        big = ctx.enter_context(tc.tile_pool(name="big", bufs=1))
        kpool = ctx.enter_context(tc.tile_pool(name="kpool", bufs=6))
        vpool = ctx.enter_context(tc.tile_pool(name="vpool", bufs=10))
        pq = ctx.enter_context(tc.tile_pool(name="pq", bufs=1, space="PSUM"))
        pcv = ctx.enter_context(tc.tile_pool(name="pcv", bufs=2, space="PSUM"))
        ptr = ctx.enter_context(tc.tile_pool(name="ptr", bufs=1, space="PSUM"))
        pat = ctx.enter_context(tc.tile_pool(name="pat", bufs=1, space="PSUM"))

        # ---- small persistent tensors ----
        q_sb = singles.tile([128, Q // 128, BC], f32r, tag="q")
        b_sb = singles.tile([128, M12], f32, tag="bias")
        kern = singles.tile([128, M12, BC], f32r, tag="kern")
        ident = singles.tile([128, 128], f32, tag="ident")
        # wr_sb shares its 24KB slot with group 0's tap staging (disjoint
        # lifetimes: wr is dead once the 48 qW matmuls finish)
        wr_sb = big.tile([128, Q // 128, KW * C], f32r, tag="wrsk")

        # Both groups pack into one tile at partition bases 0 and 32: engine
        # SBUF access is legal from base 32 for <=32 partitions, and the
        # per-partition SBUF accounting makes a [36, L] tile no bigger than
        # a [4, L] one.
        maskf = singles.tile([32 + BG, L], f32, tag="maskf")
        A_sb = singles.tile([32 + BG, L], f32, tag="a")
        ET = singles.tile([128, L // 128, BC], bf16, tag="et")
        nmx = singles.tile([32 + BG, 1], f32, tag="nmx")
        ssum = singles.tile([32 + BG, 1], f32, tag="ssum")
        sinv = singles.tile([32 + BG, 1], f32, tag="sinv")

        # params on the scalar ring: the sync ring starts streaming k at t=0
        nc.scalar.dma_start(
            out=q_sb, in_=qT[:].rearrange("(qc p) b -> p qc b", p=128).bitcast(f32r)
        )
        nc.scalar.dma_start(out=b_sb, in_=Br[:])
        wr_src = Wr[:].rearrange("(qc p) n -> p qc n", p=128).bitcast(f32r)
        for m in range(M12):
            nc.scalar.dma_start(
                out=wr_sb[:, :, m * 128:(m + 1) * 128],
                in_=wr_src[:, :, m * 128:(m + 1) * 128],
            )
        make_identity(nc, ident)

        # masks -> f32 -> additive form (m-1)*NEGBIG in {0, -NEGBIG} in place
        for g in range(G):
            pg = 32 * g
            mi = kpool.tile([BG, L], i32, tag="k")
            nc.sync.dma_start(out=mi, in_=mT[g * BG:(g + 1) * BG, :])
            nc.vector.tensor_copy(out=maskf[pg:pg + BG, :], in_=mi)
            nc.vector.tensor_scalar(
                out=maskf[pg:pg + BG, :], in0=maskf[pg:pg + BG, :],
                scalar1=-1.0, scalar2=NEGBIG,
                op0=mybir.AluOpType.add, op1=mybir.AluOpType.mult,
            )

        # ---- kern[p, m, b] = (q @ W.T + b) in (tap, channel) order ----
        for m in range(M12):
            pqt = pq.tile([128, BC], f32, tag="pq")
            for qc in range(Q // 128):
                nc.tensor.matmul(
                    pqt,
                    wr_sb[:, qc, m * 128:(m + 1) * 128],
                    q_sb[:, qc, :],
                    start=(qc == 0), stop=(qc == Q // 128 - 1),
                )
            nc.vector.tensor_scalar_add(
                out=kern[:, m, :], in0=pqt, scalar1=b_sb[:, m:m + 1]
            )
        kern_r = kern.rearrange("p (w cc) b -> p cc w b", w=KW)

        # Sk8[32g + j] rows: t_w[l] at (w, 1+l); shares wr_sb's slot (wr is
        # dead once qW finishes, before the first tap scatter lands)
        Sk8 = big.tile([32 + BG, KW, L + 3], f32, tag="wrsk")
        nc.vector.memset(Sk8[:, 0, 0:1], 0.0)          # t0[-1] = 0
        nc.vector.memset(Sk8[:, 2, L + 1:L + 2], 0.0)  # t2[L]  = 0
        Am = big.tile([32 + BG, L], f32, tag="am")     # becomes e_ij in place
        E_sb = Am

        for g in range(G):
            pg = 32 * g

            # ---- t_w[l, b] = sum_c k[l, b, c] * kern[b, c, w] (M=3) ----
            for j in range(BG):
                b_ = g * BG + j
                ksb = []
                for cc in range(C // 128):
                    kt = kpool.tile([128, L], f32r, tag="k")
                    nc.sync.dma_start(
                        out=kt, in_=kT[b_, cc * 128:(cc + 1) * 128, :].bitcast(f32r)
                    )
                    ksb.append(kt)
                scv = sa.tile([KW, L], f32, tag="scv")
                for lc in range(L // 512):
                    cv = pcv.tile([KW, 512], f32, tag="cv")
                    for cc in range(C // 128):
                        nc.tensor.matmul(
                            cv,
                            kern_r[:, cc, :, b_],
                            ksb[cc][:, lc * 512:(lc + 1) * 512],
                            start=(cc == 0), stop=(cc == C // 128 - 1),
                        )
                    nc.scalar.copy(out=scv[:, lc * 512:(lc + 1) * 512], in_=cv)
                # partition-scatter [3, L] -> partition 32g+j, scalar ring
                nc.scalar.dma_start(
                    out=Sk8[pg + j:pg + j + 1, :, 1:L + 1], in_=scv
                )

            # ---- 3-tap combine + masked softmax over l ----
            Ag = A_sb[pg:pg + BG, :]
            Amg = Am[pg:pg + BG, :]
            nc.vector.tensor_add(
                out=Ag, in0=Sk8[pg:pg + BG, 0, 0:L], in1=Sk8[pg:pg + BG, 1, 1:L + 1]
            )
            nc.vector.tensor_add(out=Ag, in0=Ag, in1=Sk8[pg:pg + BG, 2, 2:L + 2])
            nc.vector.tensor_add(out=Amg, in0=Ag, in1=maskf[pg:pg + BG, :])
            nc.vector.tensor_reduce(
                out=nmx[pg:pg + BG, :], in_=Amg, op=mybir.AluOpType.max,
                axis=mybir.AxisListType.X, negate=True,
            )
            nc.scalar.activation(
                out=Amg, in_=Amg, func=mybir.ActivationFunctionType.Exp,
                bias=nmx[pg:pg + BG, 0:1], scale=1.0,
                accum_out=ssum[pg:pg + BG, :],
            )
            nc.vector.reciprocal(out=sinv[pg:pg + BG, :], in_=ssum[pg:pg + BG, :])
            nc.vector.tensor_scalar_mul(
                out=Amg, in0=Amg, scalar1=sinv[pg:pg + BG, 0:1]
            )
            Eg = E_sb[pg:pg + BG, :]

            nc.scalar.dma_start(out=a_out[g * BG:(g + 1) * BG, :], in_=Ag)
            nc.scalar.dma_start(out=e_out[g * BG:(g + 1) * BG, :], in_=Eg)

            # ---- ET[p, lt, 4g+j] = E[j, lt*128+p] (PE transpose, cast) ----
            for lt in range(L // 128):
                trp = ptr.tile([128, BG], f32, tag="tr")
                nc.tensor.transpose(
                    trp,
                    E_sb[pg:pg + BG, lt * 128:(lt + 1) * 128],
                    ident[pg:pg + BG, pg:pg + BG],
                )
                nc.vector.tensor_copy(
                    out=ET[:, lt, g * BG:(g + 1) * BG], in_=trp
                )

            # ---- attend[b, :] = sum_l e[l, b] * v[l, b, :] ----
            # lt-outer wave: one stationary ET[:, lt, :] serves all 4 batch
            # rows (out row m pairs e(:, m) with v(:, b); row j is real).
            vsb = []
            for j in range(BG):
                b_ = g * BG + j
                pair = []
                for jj in range(2):
                    vt = vpool.tile([128, 8, V], bf16, tag="v")
                    nc.sync.dma_start(
                        out=vt,
                        in_=vT[b_, jj * 1024:(jj + 1) * 1024, :].rearrange(
                            "(i p) v -> p i v", p=128
                        ),
                    )
                    pair.append(vt)
                vsb.append(pair)
            att_ps = [pat.tile([BG, V], f32, tag=f"atp{j}", name=f"atp{j}") for j in range(BG)]
            for lt in range(L // 128):
                for j in range(BG):
                    nc.tensor.matmul(
                        att_ps[j],
                        ET[:, lt, g * BG:(g + 1) * BG],
                        vsb[j][lt // 8][:, lt % 8, :],
                        start=(lt == 0), stop=(lt == L // 128 - 1),
                    )
            for j in range(BG):
                att_sb = sa.tile([BG, V], f32, tag="attsb")
                nc.vector.tensor_copy(out=att_sb, in_=att_ps[j])
                nc.scalar.dma_start(
                    out=t_out[g * BG + j:g * BG + j + 1, :],
                    in_=att_sb[j:j + 1, :],
                )

    nc.compile()
    return nc


def get_nc():
    global _NC
    if _NC is None:
        _NC = _build()
    return _NC


def make_in_maps(q, k, v, k_mask, W, b):
    import ml_dtypes

    q = np.ascontiguousarray(np.asarray(q, dtype=np.float32))
    k = np.asarray(k, dtype=np.float32)
    v = np.asarray(v, dtype=np.float32)
    k_mask = np.asarray(k_mask, dtype=np.int32)
    W = np.asarray(W, dtype=np.float32)
    b = np.asarray(b, dtype=np.float32)

    # W column reorder: Wr[q, w*C + c] = W[c*KW + w, q]
    Wr = np.ascontiguousarray(W.reshape(C, KW, Q).transpose(2, 1, 0).reshape(Q, KW * C))
    Br = np.ascontiguousarray(b.reshape(C, KW).T.reshape(KW * C).reshape(M12, 128).T)

    in_maps = []
    for i in range(NCORES):
        sl = slice(i * BC, (i + 1) * BC)
        in_maps.append({
            "kT": np.ascontiguousarray(k[:, sl, :].transpose(1, 2, 0)),
            "vT": np.ascontiguousarray(v[:, sl, :].transpose(1, 0, 2)).astype(
                ml_dtypes.bfloat16
            ),
            "mT": np.ascontiguousarray(k_mask[:, sl].T),
            "qT": np.ascontiguousarray(q[sl, :].T),
            "Wr": Wr,
            "Br": Br,
        })
    return in_maps


def assemble(results):
    a = np.concatenate([r["a_out"] for r in results], axis=0).T.copy()
    e = np.concatenate([r["e_out"] for r in results], axis=0).T.copy()
    att = np.concatenate([r["t_out"] for r in results], axis=0)
    return (
        np.ascontiguousarray(a, dtype=np.float32),
        np.ascontiguousarray(e, dtype=np.float32),
        np.ascontiguousarray(att, dtype=np.float32),
    )


def kernel(q, k, v, k_mask, W, b):
    from concourse.bass_utils import run_bass_kernel_spmd

    nc = get_nc()
    in_maps = make_in_maps(q, k, v, k_mask, W, b)
    res = run_bass_kernel_spmd(nc, in_maps, core_ids=list(range(NCORES)))
    return assemble(res.results)


# revision 36
# speedup vs baseline: 1.0556x; 1.0556x over previous
"""Trainium2 Bass kernel for nn_AttCNN4Weight (sparse_attention).

Data-parallel over batch: each of the 8 NeuronCores handles 8 of the 64
batch elements end-to-end (dynamic per-sample conv kernel -> sliding-window
score -> masked softmax over kv_len -> weighted sum of v). No collectives.

Host-side work is layout only: batch sharding, transposes so every DMA
moves multi-KB contiguous rows, a column reorder of W to (tap, channel)
order, and a bf16 cast of v (the attend reduction tolerates bf16 easily;
halves v HBM traffic).

Performance structure (memory-bound problem, ~51MB/core HBM traffic):
- f32r single-pass matmuls for the score conv (fp32 is 2 half-rate passes).
- KW=3 taps come out of ONE matmul pass (M=3); the +/-1 tap shifts are
  resolved by free-dim offsets after a DMA partition-scatter into a
  per-batch staging row (engine SBUF access must start at partition
  0/32/64/96, so cross-partition landing goes through DMA).
- Three DMA issue rings: sync carries the big k-then-v input stream in
  demand order, scalar carries params + compute-dependent stores (so the
  input stream never head-of-line blocks on compute).
- v streams in 0.5MB tiles through a 20-deep pool so the attend phase's
  consume->release->refill pipeline stays at line rate.
"""

import sys

if "/opt/trn_rl_repo" not in sys.path:
    sys.path.insert(0, "/opt/trn_rl_repo")

import numpy as np
from contextlib import ExitStack

L, B, C, Q, V, KW = 2048, 64, 512, 512, 512, 3
NCORES = 8
BC = B // NCORES          # 8 batch elements per core
M12 = KW * (C // 128)     # 12 contraction chunks of (tap, channel)
NEGBIG = 3.0e38           # additive mask constant (finite, exp() underflows to 0)

_NC = None


def _build():
    import concourse.bacc as bacc
    import concourse.tile as tile
    from concourse import mybir
    from concourse.masks import make_identity

    f32 = mybir.dt.float32
    f32r = mybir.dt.float32r
    bf16 = mybir.dt.bfloat16
    i32 = mybir.dt.int32

    nc = bacc.Bacc(None)

    kT = nc.declare_dram_parameter("kT", [BC, C, L], f32, isOutput=False)
    vT = nc.declare_dram_parameter("vT", [BC, L, V], bf16, isOutput=False)
    mT = nc.declare_dram_parameter("mT", [BC, L], i32, isOutput=False)
    qT = nc.declare_dram_parameter("qT", [Q, BC], f32, isOutput=False)
    Wr = nc.declare_dram_parameter("Wr", [Q, KW * C], f32, isOutput=False)
    Br = nc.declare_dram_parameter("Br", [128, M12], f32, isOutput=False)
    a_out = nc.declare_dram_parameter("a_out", [BC, L], f32, isOutput=True)
    e_out = nc.declare_dram_parameter("e_out", [BC, L], f32, isOutput=True)
    t_out = nc.declare_dram_parameter("t_out", [BC, V], f32, isOutput=True)

    with ExitStack() as ctx:
        tc = ctx.enter_context(tile.TileContext(nc))
        singles = ctx.enter_context(tc.tile_pool(name="singles", bufs=1))
        sa = ctx.enter_context(tc.tile_pool(name="sa", bufs=2))
        big = ctx.enter_context(tc.tile_pool(name="big", bufs=1))
        kpool = ctx.enter_context(tc.tile_pool(name="kpool", bufs=6))
        vpool = ctx.enter_context(tc.tile_pool(name="vpool", bufs=20))
        pq = ctx.enter_context(tc.tile_pool(name="pq", bufs=1, space="PSUM"))
        pcv = ctx.enter_context(tc.tile_pool(name="pcv", bufs=2, space="PSUM"))
        ptr = ctx.enter_context(tc.tile_pool(name="ptr", bufs=2, space="PSUM"))
        pat = ctx.enter_context(tc.tile_pool(name="pat", bufs=2, space="PSUM"))

        # ---- small persistent tensors ----
        q_sb = singles.tile([128, Q // 128, BC], f32r, tag="q")
        b_sb = singles.tile([128, M12], f32, tag="bias")
        kern = singles.tile([128, M12, BC], f32r, tag="kern")
        ident = singles.tile([128, 128], f32, tag="ident")
        maskf = singles.tile([BC, L], f32, tag="maskf")
        A_sb = singles.tile([BC, L], f32, tag="a")
        ET = singles.tile([128, L // 128, BC], bf16, tag="et")
        nmx = singles.tile([BC, 1], f32, tag="nmx")
        ssum = singles.tile([BC, 1], f32, tag="ssum")
        sinv = singles.tile([BC, 1], f32, tag="sinv")
        # wr_sb / Sk8 / Am share one 24KB slot (disjoint lifetimes)
        wr_sb = big.tile([128, Q // 128, KW * C], f32r, tag="wrsk")

        # params on the scalar ring: the sync ring starts streaming k at t=0
        nc.scalar.dma_start(
            out=q_sb, in_=qT[:].rearrange("(qc p) b -> p qc b", p=128).bitcast(f32r)
        )
        nc.scalar.dma_start(out=b_sb, in_=Br[:])
        wr_src = Wr[:].rearrange("(qc p) n -> p qc n", p=128).bitcast(f32r)
        for m in range(M12):
            nc.scalar.dma_start(
                out=wr_sb[:, :, m * 128:(m + 1) * 128],
                in_=wr_src[:, :, m * 128:(m + 1) * 128],
            )
        make_identity(nc, ident)

        # mask -> f32 -> additive form (m-1)*NEGBIG in {0, -NEGBIG} in place
        msk_i = kpool.tile([BC, L], i32, tag="k")
        nc.sync.dma_start(out=msk_i, in_=mT[:])
        nc.vector.tensor_copy(out=maskf, in_=msk_i)
        nc.vector.tensor_scalar(
            out=maskf, in0=maskf, scalar1=-1.0, scalar2=NEGBIG,
            op0=mybir.AluOpType.add, op1=mybir.AluOpType.mult,
        )

        # ---- kern[p, m, b] = (q @ W.T + b) in (tap, channel) order ----
        for m in range(M12):
            pqt = pq.tile([128, BC], f32, tag="pq")
            for qc in range(Q // 128):
                nc.tensor.matmul(
                    pqt,
                    wr_sb[:, qc, m * 128:(m + 1) * 128],
                    q_sb[:, qc, :],
                    start=(qc == 0), stop=(qc == Q // 128 - 1),
                )
            nc.vector.tensor_scalar_add(
                out=kern[:, m, :], in0=pqt, scalar1=b_sb[:, m:m + 1]
            )
        kern_r = kern.rearrange("p (w cc) b -> p cc w b", w=KW)

        # ---- t_w[l, b] = sum_c k[l, b, c] * kern[b, c, w] (M=3: one rhs
        # pass computes all three taps). Taps land via DMA partition-scatter
        # into Sk8[b] = [3, L+3] rows (t_w[j] at (w, 1+j)).
        Sk8 = big.tile([BC, KW, L + 3], f32, tag="wrsk")
        nc.vector.memset(Sk8[:, 0, 0:1], 0.0)          # t0[-1] = 0
        nc.vector.memset(Sk8[:, 2, L + 1:L + 2], 0.0)  # t2[L]  = 0
        for b_ in range(BC):
            ksb = []
            for cc in range(C // 128):
                kt = kpool.tile([128, L], f32r, tag="k")
                nc.sync.dma_start(
                    out=kt, in_=kT[b_, cc * 128:(cc + 1) * 128, :].bitcast(f32r)
                )
                ksb.append(kt)
            scv = sa.tile([KW, L], f32, tag="scv")
            for lc in range(L // 512):
                cv = pcv.tile([KW, 512], f32, tag="cv")
                for cc in range(C // 128):
                    nc.tensor.matmul(
                        cv,
                        kern_r[:, cc, :, b_],
                        ksb[cc][:, lc * 512:(lc + 1) * 512],
                        start=(cc == 0), stop=(cc == C // 128 - 1),
                    )
                nc.scalar.copy(out=scv[:, lc * 512:(lc + 1) * 512], in_=cv)
            # partition-scatter [3, L] -> partition b_, on the scalar ring
            nc.scalar.dma_start(out=Sk8[b_:b_ + 1, :, 1:L + 1], in_=scv)

        # ---- 3-tap combine + masked softmax over l ----
        Am = big.tile([BC, L], f32, tag="wrsk")  # becomes e_ij in place
        nc.vector.tensor_add(
            out=A_sb, in0=Sk8[:, 0, 0:L], in1=Sk8[:, 1, 1:L + 1]
        )
        nc.vector.tensor_add(out=A_sb, in0=A_sb, in1=Sk8[:, 2, 2:L + 2])
        nc.vector.tensor_add(out=Am, in0=A_sb, in1=maskf)
        nc.vector.tensor_reduce(
            out=nmx, in_=Am, op=mybir.AluOpType.max,
            axis=mybir.AxisListType.X, negate=True,
        )
        nc.scalar.activation(
            out=Am, in_=Am, func=mybir.ActivationFunctionType.Exp,
            bias=nmx[:, 0:1], scale=1.0, accum_out=ssum,
        )
        nc.vector.reciprocal(out=sinv, in_=ssum)
        nc.vector.tensor_scalar_mul(out=Am, in0=Am, scalar1=sinv[:, 0:1])
        E_sb = Am

        nc.scalar.dma_start(out=a_out[:], in_=A_sb)
        nc.scalar.dma_start(out=e_out[:], in_=E_sb)

        # ---- ET[p, lt, b] = E[b, lt*128+p] (PE transpose, cast to bf16) ----
        for lt in range(L // 128):
            trp = ptr.tile([128, BC], f32, tag="tr")
            nc.tensor.transpose(trp, E_sb[:, lt * 128:(lt + 1) * 128], ident[0:BC, 0:BC])
            nc.vector.tensor_copy(out=ET[:, lt, :], in_=trp)

        # ---- attend[b, :] = sum_l e[l, b] * v[l, b, :] ----
        # lhsT = ET[:, lt, :] gives an [8, 512] output whose row m pairs
        # e(:, m) with v(:, b); only row b is real — DMA just that row out.
        for b_ in range(BC):
            vsb = []
            for jj in range(4):
                vt = vpool.tile([128, 4, V], bf16, tag="v")
                nc.sync.dma_start(
                    out=vt,
                    in_=vT[b_, jj * 512:(jj + 1) * 512, :].rearrange(
                        "(i p) v -> p i v", p=128
                    ),
                )
                vsb.append(vt)
            att_ps = pat.tile([BC, V], f32, tag="atp")
            for lt in range(L // 128):
                nc.tensor.matmul(
                    att_ps,
                    ET[:, lt, :],
                    vsb[lt // 4][:, lt % 4, :],
                    start=(lt == 0), stop=(lt == L // 128 - 1),
                )
            att_sb = sa.tile([BC, V], f32, tag="attsb")
            nc.vector.tensor_copy(out=att_sb, in_=att_ps)
            nc.scalar.dma_start(out=t_out[b_:b_ + 1, :], in_=att_sb[b_:b_ + 1, :])

    nc.compile()
    return nc


def get_nc():
    global _NC
    if _NC is None:
        _NC = _build()
    return _NC


def make_in_maps(q, k, v, k_mask, W, b):
    import ml_dtypes

    q = np.ascontiguousarray(np.asarray(q, dtype=np.float32))
    k = np.asarray(k, dtype=np.float32)
    v = np.asarray(v, dtype=np.float32)
    k_mask = np.asarray(k_mask, dtype=np.int32)
    W = np.asarray(W, dtype=np.float32)
    b = np.asarray(b, dtype=np.float32)

    # W column reorder: Wr[q, w*C + c] = W[c*KW + w, q]
    Wr = np.ascontiguousarray(W.reshape(C, KW, Q).transpose(2, 1, 0).reshape(Q, KW * C))
    Br = np.ascontiguousarray(b.reshape(C, KW).T.reshape(KW * C).reshape(M12, 128).T)

    in_maps = []
    for i in range(NCORES):
        sl = slice(i * BC, (i + 1) * BC)
        in_maps.append({
            "kT": np.ascontiguousarray(k[:, sl, :].transpose(1, 2, 0)),
            "vT": np.ascontiguousarray(v[:, sl, :].transpose(1, 0, 2)).astype(
                ml_dtypes.bfloat16
            ),
            "mT": np.ascontiguousarray(k_mask[:, sl].T),
            "qT": np.ascontiguousarray(q[sl, :].T),
            "Wr": Wr,
            "Br": Br,
        })
    return in_maps


def assemble(results):
    a = np.concatenate([r["a_out"] for r in results], axis=0).T.copy()
    e = np.concatenate([r["e_out"] for r in results], axis=0).T.copy()
    att = np.concatenate([r["t_out"] for r in results], axis=0)
    return (
        np.ascontiguousarray(a, dtype=np.float32),
        np.ascontiguousarray(e, dtype=np.float32),
        np.ascontiguousarray(att, dtype=np.float32),
    )


def kernel(q, k, v, k_mask, W, b):
    from concourse.bass_utils import run_bass_kernel_spmd

    nc = get_nc()
    in_maps = make_in_maps(q, k, v, k_mask, W, b)
    res = run_bass_kernel_spmd(nc, in_maps, core_ids=list(range(NCORES)))
    return assemble(res.results)


# revision 37
# speedup vs baseline: 1.1263x; 1.0670x over previous
"""Trainium2 Bass kernel for nn_AttCNN4Weight (sparse_attention).

Data-parallel over batch: each of the 8 NeuronCores handles 8 of the 64
batch elements end-to-end (dynamic per-sample conv kernel -> sliding-window
score -> masked softmax over kv_len -> weighted sum of v). No collectives.

Host-side work is layout only: batch sharding, transposes so every DMA
moves multi-KB contiguous rows, a column reorder of W to (tap, channel)
order, and a bf16 cast of v (the attend reduction tolerates bf16 easily;
halves v HBM traffic).

Performance structure (memory-bound problem, ~51MB/core HBM traffic):
- f32r single-pass matmuls for the score conv (fp32 is 2 half-rate passes).
- KW=3 taps come out of ONE matmul pass (M=3); the +/-1 tap shifts are
  resolved by free-dim offsets after a DMA partition-scatter into a
  per-batch staging row (engine SBUF access must start at partition
  0/32/64/96, so cross-partition landing goes through DMA).
- Three DMA issue rings: sync carries the big k-then-v input stream in
  demand order, scalar carries params + compute-dependent stores (so the
  input stream never head-of-line blocks on compute).
- v streams in 0.5MB tiles through a 20-deep pool so the attend phase's
  consume->release->refill pipeline stays at line rate.
"""

import sys

if "/opt/trn_rl_repo" not in sys.path:
    sys.path.insert(0, "/opt/trn_rl_repo")

import numpy as np
from contextlib import ExitStack

L, B, C, Q, V, KW = 2048, 64, 512, 512, 512, 3
NCORES = 8
BC = B // NCORES          # 8 batch elements per core
M12 = KW * (C // 128)     # 12 contraction chunks of (tap, channel)
NEGBIG = 3.0e38           # additive mask constant (finite, exp() underflows to 0)

_NC = None


def _build():
    import concourse.bacc as bacc
    import concourse.tile as tile
    from concourse import mybir
    from concourse.masks import make_identity

    f32 = mybir.dt.float32
    f32r = mybir.dt.float32r
    bf16 = mybir.dt.bfloat16
    i32 = mybir.dt.int32

    nc = bacc.Bacc(None)

    kT = nc.declare_dram_parameter("kT", [BC, C, L], f32, isOutput=False)
    vT = nc.declare_dram_parameter("vT", [BC, L, V], bf16, isOutput=False)
    mT = nc.declare_dram_parameter("mT", [BC, L], i32, isOutput=False)
    qT = nc.declare_dram_parameter("qT", [Q, BC], f32, isOutput=False)
    Wr = nc.declare_dram_parameter("Wr", [Q, KW * C], f32, isOutput=False)
    Br = nc.declare_dram_parameter("Br", [128, M12], f32, isOutput=False)
    a_out = nc.declare_dram_parameter("a_out", [BC, L], f32, isOutput=True)
    e_out = nc.declare_dram_parameter("e_out", [BC, L], f32, isOutput=True)
    t_out = nc.declare_dram_parameter("t_out", [BC, V], f32, isOutput=True)

    with ExitStack() as ctx:
        tc = ctx.enter_context(tile.TileContext(nc))
        singles = ctx.enter_context(tc.tile_pool(name="singles", bufs=1))
        sa = ctx.enter_context(tc.tile_pool(name="sa", bufs=2))
        big = ctx.enter_context(tc.tile_pool(name="big", bufs=1))
        kpool = ctx.enter_context(tc.tile_pool(name="kpool", bufs=6))
        vpool = ctx.enter_context(tc.tile_pool(name="vpool", bufs=12))
        pq = ctx.enter_context(tc.tile_pool(name="pq", bufs=1, space="PSUM"))
        pcv = ctx.enter_context(tc.tile_pool(name="pcv", bufs=2, space="PSUM"))
        ptr = ctx.enter_context(tc.tile_pool(name="ptr", bufs=2, space="PSUM"))
        pat = ctx.enter_context(tc.tile_pool(name="pat", bufs=2, space="PSUM"))

        # ---- small persistent tensors ----
        q_sb = singles.tile([128, Q // 128, BC], f32r, tag="q")
        b_sb = singles.tile([128, M12], f32, tag="bias")
        kern = singles.tile([128, M12, BC], f32r, tag="kern")
        ident = singles.tile([128, 128], f32, tag="ident")
        maskf = singles.tile([BC, L], f32, tag="maskf")
        A_sb = singles.tile([BC, L], f32, tag="a")
        ET = singles.tile([128, L // 128, BC], bf16, tag="et")
        nmx = singles.tile([BC, 1], f32, tag="nmx")
        ssum = singles.tile([BC, 1], f32, tag="ssum")
        sinv = singles.tile([BC, 1], f32, tag="sinv")
        # wr_sb / Sk8 / Am share one 24KB slot (disjoint lifetimes)
        wr_sb = big.tile([128, Q // 128, KW * C], f32r, tag="wrsk")

        # params on the scalar ring: the sync ring starts streaming k at t=0
        nc.scalar.dma_start(
            out=q_sb, in_=qT[:].rearrange("(qc p) b -> p qc b", p=128).bitcast(f32r)
        )
        nc.scalar.dma_start(out=b_sb, in_=Br[:])
        wr_src = Wr[:].rearrange("(qc p) n -> p qc n", p=128).bitcast(f32r)
        for m in range(M12):
            nc.scalar.dma_start(
                out=wr_sb[:, :, m * 128:(m + 1) * 128],
                in_=wr_src[:, :, m * 128:(m + 1) * 128],
            )
        make_identity(nc, ident)

        # mask -> f32 -> additive form (m-1)*NEGBIG in {0, -NEGBIG} in place
        msk_i = kpool.tile([BC, L], i32, tag="k")
        nc.sync.dma_start(out=msk_i, in_=mT[:])
        nc.vector.tensor_copy(out=maskf, in_=msk_i)
        nc.vector.tensor_scalar(
            out=maskf, in0=maskf, scalar1=-1.0, scalar2=NEGBIG,
            op0=mybir.AluOpType.add, op1=mybir.AluOpType.mult,
        )

        # ---- kern[p, m, b] = (q @ W.T + b) in (tap, channel) order ----
        for m in range(M12):
            pqt = pq.tile([128, BC], f32, tag="pq")
            for qc in range(Q // 128):
                nc.tensor.matmul(
                    pqt,
                    wr_sb[:, qc, m * 128:(m + 1) * 128],
                    q_sb[:, qc, :],
                    start=(qc == 0), stop=(qc == Q // 128 - 1),
                )
            nc.vector.tensor_scalar_add(
                out=kern[:, m, :], in0=pqt, scalar1=b_sb[:, m:m + 1]
            )
        kern_r = kern.rearrange("p (w cc) b -> p cc w b", w=KW)

        # ---- t_w[l, b] = sum_c k[l, b, c] * kern[b, c, w] (M=3: one rhs
        # pass computes all three taps). Taps land via DMA partition-scatter
        # into Sk8[b] = [3, L+3] rows (t_w[j] at (w, 1+j)).
        Sk8 = big.tile([BC, KW, L + 3], f32, tag="wrsk")
        nc.vector.memset(Sk8[:, 0, 0:1], 0.0)          # t0[-1] = 0
        nc.vector.memset(Sk8[:, 2, L + 1:L + 2], 0.0)  # t2[L]  = 0
        for b_ in range(BC):
            ksb = []
            for cc in range(C // 128):
                kt = kpool.tile([128, L], f32r, tag="k")
                nc.sync.dma_start(
                    out=kt, in_=kT[b_, cc * 128:(cc + 1) * 128, :].bitcast(f32r)
                )
                ksb.append(kt)
            scv = sa.tile([KW, L], f32, tag="scv")
            for lc in range(L // 512):
                cv = pcv.tile([KW, 512], f32, tag="cv")
                for cc in range(C // 128):
                    nc.tensor.matmul(
                        cv,
                        kern_r[:, cc, :, b_],
                        ksb[cc][:, lc * 512:(lc + 1) * 512],
                        start=(cc == 0), stop=(cc == C // 128 - 1),
                    )
                nc.scalar.copy(out=scv[:, lc * 512:(lc + 1) * 512], in_=cv)
            # partition-scatter [3, L] -> partition b_, on the scalar ring
            nc.scalar.dma_start(out=Sk8[b_:b_ + 1, :, 1:L + 1], in_=scv)

        # ---- 3-tap combine + masked softmax over l ----
        Am = big.tile([BC, L], f32, tag="wrsk")  # becomes e_ij in place
        nc.vector.tensor_add(
            out=A_sb, in0=Sk8[:, 0, 0:L], in1=Sk8[:, 1, 1:L + 1]
        )
        nc.vector.tensor_add(out=A_sb, in0=A_sb, in1=Sk8[:, 2, 2:L + 2])
        nc.vector.tensor_add(out=Am, in0=A_sb, in1=maskf)
        nc.vector.tensor_reduce(
            out=nmx, in_=Am, op=mybir.AluOpType.max,
            axis=mybir.AxisListType.X, negate=True,
        )
        nc.scalar.activation(
            out=Am, in_=Am, func=mybir.ActivationFunctionType.Exp,
            bias=nmx[:, 0:1], scale=1.0, accum_out=ssum,
        )
        nc.vector.reciprocal(out=sinv, in_=ssum)
        nc.vector.tensor_scalar_mul(out=Am, in0=Am, scalar1=sinv[:, 0:1])
        E_sb = Am

        nc.scalar.dma_start(out=a_out[:], in_=A_sb)
        nc.scalar.dma_start(out=e_out[:], in_=E_sb)

        # ---- ET[p, lt, b] = E[b, lt*128+p] (PE transpose, cast to bf16) ----
        for lt in range(L // 128):
            trp = ptr.tile([128, BC], f32, tag="tr")
            nc.tensor.transpose(trp, E_sb[:, lt * 128:(lt + 1) * 128], ident[0:BC, 0:BC])
            nc.vector.tensor_copy(out=ET[:, lt, :], in_=trp)

        # ---- attend[b, :] = sum_l e[l, b] * v[l, b, :] ----
        # lhsT = ET[:, lt, :] gives an [8, 512] output whose row m pairs
        # e(:, m) with v(:, b); only row b is real — DMA just that row out.
        for b_ in range(BC):
            vsb = []
            for jj in range(2):
                vt = vpool.tile([128, 8, V], bf16, tag="v")
                nc.sync.dma_start(
                    out=vt,
                    in_=vT[b_, jj * 1024:(jj + 1) * 1024, :].rearrange(
                        "(i p) v -> p i v", p=128
                    ),
                )
                vsb.append(vt)
            att_ps = pat.tile([BC, V], f32, tag="atp")
            for lt in range(L // 128):
                nc.tensor.matmul(
                    att_ps,
                    ET[:, lt, :],
                    vsb[lt // 8][:, lt % 8, :],
                    start=(lt == 0), stop=(lt == L // 128 - 1),
                )
            att_sb = sa.tile([BC, V], f32, tag="attsb")
            nc.vector.tensor_copy(out=att_sb, in_=att_ps)
            nc.scalar.dma_start(out=t_out[b_:b_ + 1, :], in_=att_sb[b_:b_ + 1, :])

    nc.compile()
    return nc


def get_nc():
    global _NC
    if _NC is None:
        _NC = _build()
    return _NC


def make_in_maps(q, k, v, k_mask, W, b):
    import ml_dtypes

    q = np.ascontiguousarray(np.asarray(q, dtype=np.float32))
    k = np.asarray(k, dtype=np.float32)
    v = np.asarray(v, dtype=np.float32)
    k_mask = np.asarray(k_mask, dtype=np.int32)
    W = np.asarray(W, dtype=np.float32)
    b = np.asarray(b, dtype=np.float32)

    # W column reorder: Wr[q, w*C + c] = W[c*KW + w, q]
    Wr = np.ascontiguousarray(W.reshape(C, KW, Q).transpose(2, 1, 0).reshape(Q, KW * C))
    Br = np.ascontiguousarray(b.reshape(C, KW).T.reshape(KW * C).reshape(M12, 128).T)

    in_maps = []
    for i in range(NCORES):
        sl = slice(i * BC, (i + 1) * BC)
        in_maps.append({
            "kT": np.ascontiguousarray(k[:, sl, :].transpose(1, 2, 0)),
            "vT": np.ascontiguousarray(v[:, sl, :].transpose(1, 0, 2)).astype(
                ml_dtypes.bfloat16
            ),
            "mT": np.ascontiguousarray(k_mask[:, sl].T),
            "qT": np.ascontiguousarray(q[sl, :].T),
            "Wr": Wr,
            "Br": Br,
        })
    return in_maps


def assemble(results):
    a = np.concatenate([r["a_out"] for r in results], axis=0).T.copy()
    e = np.concatenate([r["e_out"] for r in results], axis=0).T.copy()
    att = np.concatenate([r["t_out"] for r in results], axis=0)
    return (
        np.ascontiguousarray(a, dtype=np.float32),
        np.ascontiguousarray(e, dtype=np.float32),
        np.ascontiguousarray(att, dtype=np.float32),
    )


def kernel(q, k, v, k_mask, W, b):
    from concourse.bass_utils import run_bass_kernel_spmd

    nc = get_nc()
    in_maps = make_in_maps(q, k, v, k_mask, W, b)
    res = run_bass_kernel_spmd(nc, in_maps, core_ids=list(range(NCORES)))
    return assemble(res.results)


# revision 38
# speedup vs baseline: 1.1868x; 1.0536x over previous
"""Trainium2 Bass kernel for nn_AttCNN4Weight (sparse_attention).

Data-parallel over batch: each of the 8 NeuronCores handles 8 of the 64
batch elements end-to-end (dynamic per-sample conv kernel -> sliding-window
score -> masked softmax over kv_len -> weighted sum of v). No collectives.

Host-side work is layout only: batch sharding, transposes so every DMA
moves multi-KB contiguous rows, a column reorder of W to (tap, channel)
order, and a bf16 cast of v (the attend reduction tolerates bf16 easily;
halves v HBM traffic).

Performance structure (memory-bound problem, ~51MB/core HBM traffic):
- f32r single-pass matmuls for the score conv (fp32 is 2 half-rate passes).
- KW=3 taps come out of ONE matmul pass (M=3); the +/-1 tap shifts are
  resolved by free-dim offsets after a DMA partition-scatter into a
  per-batch staging row (engine SBUF access must start at partition
  0/32/64/96, so cross-partition landing goes through DMA).
- Three DMA issue rings: sync carries the big k-then-v input stream in
  demand order, scalar carries params + compute-dependent stores (so the
  input stream never head-of-line blocks on compute).
- v streams in 0.5MB tiles through a 20-deep pool so the attend phase's
  consume->release->refill pipeline stays at line rate.
"""

import sys

if "/opt/trn_rl_repo" not in sys.path:
    sys.path.insert(0, "/opt/trn_rl_repo")

import numpy as np
from contextlib import ExitStack

L, B, C, Q, V, KW = 2048, 64, 512, 512, 512, 3
NCORES = 8
BC = B // NCORES          # 8 batch elements per core
M12 = KW * (C // 128)     # 12 contraction chunks of (tap, channel)
NEGBIG = 3.0e38           # additive mask constant (finite, exp() underflows to 0)

_NC = None


def _build():
    import concourse.bacc as bacc
    import concourse.tile as tile
    from concourse import mybir
    from concourse.masks import make_identity

    f32 = mybir.dt.float32
    f32r = mybir.dt.float32r
    bf16 = mybir.dt.bfloat16
    i32 = mybir.dt.int32

    nc = bacc.Bacc(None)

    kT = nc.declare_dram_parameter("kT", [BC, C, L], f32, isOutput=False)
    vT = nc.declare_dram_parameter("vT", [BC, L, V], bf16, isOutput=False)
    mT = nc.declare_dram_parameter("mT", [BC, L], i32, isOutput=False)
    qT = nc.declare_dram_parameter("qT", [Q, BC], f32, isOutput=False)
    Wr = nc.declare_dram_parameter("Wr", [Q, KW * C], f32, isOutput=False)
    Br = nc.declare_dram_parameter("Br", [128, M12], f32, isOutput=False)
    a_out = nc.declare_dram_parameter("a_out", [BC, L], f32, isOutput=True)
    e_out = nc.declare_dram_parameter("e_out", [BC, L], f32, isOutput=True)
    t_out = nc.declare_dram_parameter("t_out", [BC, V], f32, isOutput=True)

    with ExitStack() as ctx:
        tc = ctx.enter_context(tile.TileContext(nc))
        singles = ctx.enter_context(tc.tile_pool(name="singles", bufs=1))
        sa = ctx.enter_context(tc.tile_pool(name="sa", bufs=2))
        big = ctx.enter_context(tc.tile_pool(name="big", bufs=1))
        kpool = ctx.enter_context(tc.tile_pool(name="kpool", bufs=6))
        vpool = ctx.enter_context(tc.tile_pool(name="vpool", bufs=12))
        pq = ctx.enter_context(tc.tile_pool(name="pq", bufs=1, space="PSUM"))
        pcv = ctx.enter_context(tc.tile_pool(name="pcv", bufs=4, space="PSUM"))
        ptr = ctx.enter_context(tc.tile_pool(name="ptr", bufs=1, space="PSUM"))
        pat = ctx.enter_context(tc.tile_pool(name="pat", bufs=2, space="PSUM"))

        # ---- small persistent tensors ----
        q_sb = singles.tile([128, Q // 128, BC], f32r, tag="q")
        b_sb = singles.tile([128, M12], f32, tag="bias")
        kern = singles.tile([128, M12, BC], f32r, tag="kern")
        ident = singles.tile([128, 128], f32, tag="ident")
        maskf = singles.tile([BC, L], f32, tag="maskf")
        A_sb = singles.tile([BC, L], f32, tag="a")
        ET = singles.tile([128, L // 128, BC], bf16, tag="et")
        nmx = singles.tile([BC, 1], f32, tag="nmx")
        ssum = singles.tile([BC, 1], f32, tag="ssum")
        sinv = singles.tile([BC, 1], f32, tag="sinv")
        # wr_sb / Sk8 / Am share one 24KB slot (disjoint lifetimes)
        wr_sb = big.tile([128, Q // 128, KW * C], f32r, tag="wrsk")

        # Wr arrives chunk-by-chunk so qW (and then conv) can start before
        # the full 3MB lands
        nc.sync.dma_start(
            out=q_sb, in_=qT[:].rearrange("(qc p) b -> p qc b", p=128).bitcast(f32r)
        )
        nc.sync.dma_start(out=b_sb, in_=Br[:])
        wr_src = Wr[:].rearrange("(qc p) n -> p qc n", p=128).bitcast(f32r)
        for m in range(M12):
            nc.sync.dma_start(
                out=wr_sb[:, :, m * 128:(m + 1) * 128],
                in_=wr_src[:, :, m * 128:(m + 1) * 128],
            )
        make_identity(nc, ident)

        # mask -> f32 -> additive form (m-1)*NEGBIG in {0, -NEGBIG} in place
        msk_i = kpool.tile([BC, L], i32, tag="k")
        nc.sync.dma_start(out=msk_i, in_=mT[:])
        nc.vector.tensor_copy(out=maskf, in_=msk_i)
        nc.vector.tensor_scalar(
            out=maskf, in0=maskf, scalar1=-1.0, scalar2=NEGBIG,
            op0=mybir.AluOpType.add, op1=mybir.AluOpType.mult,
        )

        # ---- kern[p, m, b] = (q @ W.T + b) in (tap, channel) order ----
        for m in range(M12):
            pqt = pq.tile([128, BC], f32, tag="pq")
            for qc in range(Q // 128):
                nc.tensor.matmul(
                    pqt,
                    wr_sb[:, qc, m * 128:(m + 1) * 128],
                    q_sb[:, qc, :],
                    start=(qc == 0), stop=(qc == Q // 128 - 1),
                )
            nc.vector.tensor_scalar_add(
                out=kern[:, m, :], in0=pqt, scalar1=b_sb[:, m:m + 1]
            )
        kern_r = kern.rearrange("p (w cc) b -> p cc w b", w=KW)

        # ---- t_w[l, b] = sum_c k[l, b, c] * kern[b, c, w] (M=3: one rhs
        # pass computes all three taps). Taps land via DMA partition-scatter
        # into Sk8[b] = [3, L+3] rows (t_w[j] at (w, 1+j)).
        Sk8 = big.tile([BC, KW, L + 3], f32, tag="wrsk")
        nc.vector.memset(Sk8[:, 0, 0:1], 0.0)          # t0[-1] = 0
        nc.vector.memset(Sk8[:, 2, L + 1:L + 2], 0.0)  # t2[L]  = 0
        for b_ in range(BC):
            ksb = []
            for cc in range(C // 128):
                kt = kpool.tile([128, L], f32r, tag="k")
                nc.sync.dma_start(
                    out=kt, in_=kT[b_, cc * 128:(cc + 1) * 128, :].bitcast(f32r)
                )
                ksb.append(kt)
            scv = sa.tile([KW, L], f32, tag="scv")
            for lc in range(L // 512):
                cv = pcv.tile([KW, 512], f32, tag="cv")
                for cc in range(C // 128):
                    nc.tensor.matmul(
                        cv,
                        kern_r[:, cc, :, b_],
                        ksb[cc][:, lc * 512:(lc + 1) * 512],
                        start=(cc == 0), stop=(cc == C // 128 - 1),
                    )
                nc.scalar.copy(out=scv[:, lc * 512:(lc + 1) * 512], in_=cv)
            # partition-scatter [3, L] -> partition b_, on the scalar ring
            nc.scalar.dma_start(out=Sk8[b_:b_ + 1, :, 1:L + 1], in_=scv)

        # ---- 3-tap combine + masked softmax over l ----
        Am = big.tile([BC, L], f32, tag="wrsk")  # becomes e_ij in place
        nc.vector.tensor_add(
            out=A_sb, in0=Sk8[:, 0, 0:L], in1=Sk8[:, 1, 1:L + 1]
        )
        nc.vector.tensor_add(out=A_sb, in0=A_sb, in1=Sk8[:, 2, 2:L + 2])
        nc.vector.tensor_add(out=Am, in0=A_sb, in1=maskf)
        nc.vector.tensor_reduce(
            out=nmx, in_=Am, op=mybir.AluOpType.max,
            axis=mybir.AxisListType.X, negate=True,
        )
        nc.scalar.activation(
            out=Am, in_=Am, func=mybir.ActivationFunctionType.Exp,
            bias=nmx[:, 0:1], scale=1.0, accum_out=ssum,
        )
        nc.vector.reciprocal(out=sinv, in_=ssum)
        nc.vector.tensor_scalar_mul(out=Am, in0=Am, scalar1=sinv[:, 0:1])
        E_sb = Am

        nc.scalar.dma_start(out=a_out[:], in_=A_sb)
        nc.scalar.dma_start(out=e_out[:], in_=E_sb)

        # ---- ET[p, lt, b] = E[b, lt*128+p] (PE transpose, cast to bf16) ----
        for lt in range(L // 128):
            trp = ptr.tile([128, BC], f32, tag="tr")
            nc.tensor.transpose(trp, E_sb[:, lt * 128:(lt + 1) * 128], ident[0:BC, 0:BC])
            nc.vector.tensor_copy(out=ET[:, lt, :], in_=trp)

        # ---- attend[b, :] = sum_l e[l, b] * v[l, b, :] ----
        # lhsT = ET[:, lt, :] gives an [8, 512] output whose row m pairs
        # e(:, m) with v(:, b); only row b is real — DMA just that row out.
        for b_ in range(BC):
            vsb = []
            for jj in range(2):
                vt = vpool.tile([128, 8, V], bf16, tag="v")
                nc.sync.dma_start(
                    out=vt,
                    in_=vT[b_, jj * 1024:(jj + 1) * 1024, :].rearrange(
                        "(i p) v -> p i v", p=128
                    ),
                )
                vsb.append(vt)
            att_ps = pat.tile([BC, V], f32, tag="atp")
            for lt in range(L // 128):
                nc.tensor.matmul(
                    att_ps,
                    ET[:, lt, :],
                    vsb[lt // 8][:, lt % 8, :],
                    start=(lt == 0), stop=(lt == L // 128 - 1),
                )
            att_sb = sa.tile([BC, V], f32, tag="attsb")
            nc.vector.tensor_copy(out=att_sb, in_=att_ps)
            nc.scalar.dma_start(out=t_out[b_:b_ + 1, :], in_=att_sb[b_:b_ + 1, :])

    nc.compile()
    return nc


def get_nc():
    global _NC
    if _NC is None:
        _NC = _build()
    return _NC


def make_in_maps(q, k, v, k_mask, W, b):
    import ml_dtypes

    q = np.ascontiguousarray(np.asarray(q, dtype=np.float32))
    k = np.asarray(k, dtype=np.float32)
    v = np.asarray(v, dtype=np.float32)
    k_mask = np.asarray(k_mask, dtype=np.int32)
    W = np.asarray(W, dtype=np.float32)
    b = np.asarray(b, dtype=np.float32)

    # W column reorder: Wr[q, w*C + c] = W[c*KW + w, q]
    Wr = np.ascontiguousarray(W.reshape(C, KW, Q).transpose(2, 1, 0).reshape(Q, KW * C))
    Br = np.ascontiguousarray(b.reshape(C, KW).T.reshape(KW * C).reshape(M12, 128).T)

    in_maps = []
    for i in range(NCORES):
        sl = slice(i * BC, (i + 1) * BC)
        in_maps.append({
            "kT": np.ascontiguousarray(k[:, sl, :].transpose(1, 2, 0)),
            "vT": np.ascontiguousarray(v[:, sl, :].transpose(1, 0, 2)).astype(
                ml_dtypes.bfloat16
            ),
            "mT": np.ascontiguousarray(k_mask[:, sl].T),
            "qT": np.ascontiguousarray(q[sl, :].T),
            "Wr": Wr,
            "Br": Br,
        })
    return in_maps


def assemble(results):
    a = np.concatenate([r["a_out"] for r in results], axis=0).T.copy()
    e = np.concatenate([r["e_out"] for r in results], axis=0).T.copy()
    att = np.concatenate([r["t_out"] for r in results], axis=0)
    return (
        np.ascontiguousarray(a, dtype=np.float32),
        np.ascontiguousarray(e, dtype=np.float32),
        np.ascontiguousarray(att, dtype=np.float32),
    )


def kernel(q, k, v, k_mask, W, b):
    from concourse.bass_utils import run_bass_kernel_spmd

    nc = get_nc()
    in_maps = make_in_maps(q, k, v, k_mask, W, b)
    res = run_bass_kernel_spmd(nc, in_maps, core_ids=list(range(NCORES)))
    return assemble(res.results)
